# revision 1
# baseline (speedup 1.0000x reference)
"""Trainium2 Bass kernel for nn_LossTDSurv (survival loss over hazards).

Strategy: pure data-parallel over 8 cores, plus HOST-side row grouping.
The loss is row-permutation invariant, and sharding is free-form, so the
host deals the B=524288 rows into 8 cores x 64 groups, where group v
holds only rows with idx == v (fixed 1152-row slots, padded with dummy
rows h=1e-30, e=0 whose contribution to every partial sum is exactly 0).

On device, idx is then a COMPILE-TIME constant per group, so the three
data-dependent quantities per row become static-slice operations:
   A  = sum_{k<=v-2} log(1-h_k)   -> tensor_reduce over lg[:, :, :v-1]
   C  = A + lg[v-1] + lg[v]       -> two small adds
   hv = h[v], lgv = lg[v]         -> strided copies
No gather / scan / masks / GPSIMD anywhere.  The kernel is a plain
DMA -> ACT(Ln) -> reduce pipeline at the HBM roofline.

Per-core output: [128, 6] fp32 partial sums; host combines in float64:
   loss = 0.5*L_z + 0.5*L_c + 1.0*nll
"""

import numpy as np

B_TOTAL = 524288
T = 64
N_CORES = 8
G = 64                 # idx groups
JB = 9                 # row-blocks of 128 per group -> 1152 rows/group
GR = 128 * JB          # rows per group slot
RPC = G * GR           # padded rows per core = 73728
GPST = 8               # groups per supertile (ACT batching)
H_DUMMY = 1e-30
LOG_CLIP = float(np.log(np.float32(1e-8)))

_CACHE = {}


def _build_nc(jb=JB, gpst=GPST):
    """Single-core SPMD Bass program (same NEFF on all 8 cores)."""
    import concourse.bacc as bacc
    import concourse.mybir as mybir
    import concourse.tile as tile

    gr = 128 * jb
    fwg = jb * T                    # free width of one group = jb*64
    nbuf = G * jb                   # per-row buffer width = 576
    f32 = mybir.dt.float32
    AF = mybir.ActivationFunctionType
    OP = mybir.AluOpType
    AX = mybir.AxisListType

    nc = bacc.Bacc("TRN2", target_bir_lowering=False, debug=False)

    hsort = nc.dram_tensor("hsort", [G * gr, T], f32, kind="ExternalInput")
    esort = nc.dram_tensor("esort", [128, nbuf], f32, kind="ExternalInput")
    partials = nc.dram_tensor("partials", [128, 12], f32, kind="ExternalOutput")

    with tile.TileContext(nc) as tc:
        with (
            tc.tile_pool(name="io", bufs=4) as io,
            tc.tile_pool(name="work", bufs=2) as work,
            tc.tile_pool(name="pers", bufs=1) as pers,
        ):
            Ab = pers.tile([128, nbuf], f32, tag="Ab")
            Cb = pers.tile([128, nbuf], f32, tag="Cb")
            Hb = pers.tile([128, nbuf], f32, tag="Hb")
            Eb = pers.tile([128, nbuf], f32, tag="Eb")
            nc.sync.dma_start(Eb[:], esort[:])

            n_st = G // gpst
            for st in range(n_st):
                g0 = st * gpst
                # contiguous [gpst*gr rows, 64] -> [128, gpst*fwg]
                Wt = io.tile([128, gpst * fwg], f32, tag="W")
                hview = hsort[g0 * gr:(g0 + gpst) * gr, :].rearrange(
                    "(g p j) t -> p g (j t)", p=128, g=gpst
                )
                nc.sync.dma_start(
                    Wt[:].rearrange("p (g f) -> p g f", g=gpst), hview
                )
                lg = work.tile([128, gpst * fwg], f32, tag="lg")
                nc.scalar.activation(lg[:], Wt[:], AF.Ln, bias=1.0, scale=-1.0)

                lg4 = lg[:].rearrange("p (g j t) -> p g j t", g=gpst, t=T)
                w4 = Wt[:].rearrange("p (g j t) -> p g j t", g=gpst, t=T)
                for gi in range(gpst):
                    v = g0 + gi
                    sl = slice(v * jb, (v + 1) * jb)
                    if v >= 2:
                        nc.vector.tensor_reduce(
                            Ab[:, sl], lg4[:, gi, :, :v - 1], axis=AX.X, op=OP.add
                        )
                    else:
                        nc.vector.memset(Ab[:, sl], 0.0)
                    # C = A + lg[v-1] + lg[v]
                    if v == 0:
                        nc.vector.tensor_copy(Cb[:, sl], lg4[:, gi, :, 0])
                    elif v == 1:
                        nc.vector.tensor_tensor(
                            out=Cb[:, sl], in0=lg4[:, gi, :, 0],
                            in1=lg4[:, gi, :, 1], op=OP.add,
                        )
                    else:
                        nc.vector.tensor_tensor(
                            out=Cb[:, sl], in0=Ab[:, sl],
                            in1=lg4[:, gi, :, v - 1], op=OP.add,
                        )
                        nc.vector.tensor_tensor(
                            out=Cb[:, sl], in0=Cb[:, sl],
                            in1=lg4[:, gi, :, v], op=OP.add,
                        )
                    nc.vector.tensor_copy(Hb[:, sl], w4[:, gi, :, v])

            # ---------------- epilogue (two halves, first overlaps loop) ---
            ep = pers.tile([128, nbuf], f32, tag="ep")
            ep2 = pers.tile([128, nbuf], f32, tag="ep2")
            acc = pers.tile([128, 12], f32, tag="acc")
            loghv = pers.tile([128, nbuf], f32, tag="loghv")
            lgv = pers.tile([128, nbuf], f32, tag="lgv")
            logwt = pers.tile([128, nbuf], f32, tag="logwt")

            def epilogue(half):
                lo, hi = half * (nbuf // 2), (half + 1) * (nbuf // 2)
                hs = slice(lo, hi)
                a0 = 6 * half
                nc.scalar.activation(loghv[:, hs], Hb[:, hs], AF.Ln)
                # s0 = sum e*(loghv + A)
                nc.vector.tensor_tensor(out=ep[:, hs], in0=loghv[:, hs],
                                        in1=Ab[:, hs], op=OP.add)
                nc.vector.scalar_tensor_tensor(
                    out=ep2[:, hs], in0=ep[:, hs], scalar=0.0, in1=Eb[:, hs],
                    op0=OP.add, op1=OP.mult, accum_out=acc[:, a0:a0 + 1],
                )
                # s1 = sum e
                nc.vector.tensor_reduce(acc[:, a0 + 1:a0 + 2], Eb[:, hs],
                                        axis=AX.X, op=OP.add)
                # censoring: s2 = sum A ; s3 = sum e*(logwt - A)
                nc.scalar.activation(ep[:, hs], Ab[:, hs], AF.Exp)
                nc.vector.tensor_scalar(
                    out=ep2[:, hs], in0=ep[:, hs], scalar1=-1.0, scalar2=1.0,
                    op0=OP.mult, op1=OP.add,
                )  # 1 - exp(A)
                nc.vector.tensor_scalar_max(out=ep2[:, hs], in0=ep2[:, hs],
                                            scalar1=1e-8)
                nc.scalar.activation(logwt[:, hs], ep2[:, hs], AF.Ln)
                if half == 0:
                    # groups v=0,1: reference gives log(1e-8) exactly
                    nc.vector.memset(logwt[:, 0:2 * jb], LOG_CLIP)
                nc.vector.tensor_reduce(acc[:, a0 + 2:a0 + 3], Ab[:, hs],
                                        axis=AX.X, op=OP.add)
                nc.vector.tensor_tensor(out=ep[:, hs], in0=logwt[:, hs],
                                        in1=Ab[:, hs], op=OP.subtract)
                nc.vector.scalar_tensor_tensor(
                    out=ep2[:, hs], in0=ep[:, hs], scalar=0.0, in1=Eb[:, hs],
                    op0=OP.add, op1=OP.mult, accum_out=acc[:, a0 + 3:a0 + 4],
                )
                # nll: s4 = sum C ; s5 = sum e*phi, phi = loghv - ln(1-hv)
                nc.vector.tensor_reduce(acc[:, a0 + 4:a0 + 5], Cb[:, hs],
                                        axis=AX.X, op=OP.add)
                nc.scalar.activation(lgv[:, hs], Hb[:, hs], AF.Ln,
                                     bias=1.0, scale=-1.0)
                nc.vector.tensor_tensor(out=ep[:, hs], in0=loghv[:, hs],
                                        in1=lgv[:, hs], op=OP.subtract)
                nc.vector.scalar_tensor_tensor(
                    out=ep[:, hs], in0=ep[:, hs], scalar=0.0, in1=Eb[:, hs],
                    op0=OP.add, op1=OP.mult, accum_out=acc[:, a0 + 5:a0 + 6],
                )

            epilogue(0)
            epilogue(1)

            nc.sync.dma_start(partials[:], acc[:])

    nc.finalize()
    return nc


def _pack_core(preds_rows, e_rows, idx_rows, jb=JB):
    """Pack one core's rows into the grouped layout.

    Returns hsort [G*gr, T] and esort [128, G*jb]."""
    gr = 128 * jb
    hsort = np.full((G * gr, T), H_DUMMY, np.float32)
    e_slot = np.zeros(G * gr, np.float32)
    for v in range(G):
        m = idx_rows == v
        n = int(m.sum())
        assert n <= gr, f"group {v} overflow: {n} > {gr}"
        hsort[v * gr:v * gr + n] = preds_rows[m]
        e_slot[v * gr:v * gr + n] = e_rows[m]
    esort = (
        e_slot.reshape(G, 128, jb).transpose(1, 0, 2).reshape(128, G * jb)
    )
    return hsort, np.ascontiguousarray(esort)


def _combine(partials_list, b_total):
    s = np.zeros(12, np.float64)
    for pcore in partials_list:
        s += pcore.astype(np.float64).sum(axis=0)
    s = s[:6] + s[6:]
    s_eu, s_e, s_a, s_ed, s_c, s_ephi = s
    L_z = -s_eu / s_e
    L_c = -(s_a + s_ed) / b_total
    nll = -(s_c + s_ephi) / b_total
    return np.float32(0.5 * L_z + 0.5 * L_c + 1.0 * nll)


def kernel(preds: np.ndarray, target: np.ndarray) -> np.ndarray:
    from concourse.bass_utils import run_bass_kernel_spmd

    preds = np.asarray(preds, np.float32).reshape(B_TOTAL, T)
    target = np.asarray(target, np.float32).reshape(B_TOTAL, 3)
    idx = target[:, 0].astype(np.int64)
    ev = target[:, 1].astype(np.float32)

    if "nc" not in _CACHE:
        _CACHE["nc"] = _build_nc()
    nc = _CACHE["nc"]

    # deal rows round-robin across cores (keeps every per-core idx-group
    # below its fixed 1152-row slot with overwhelming probability)
    in_maps = []
    for c in range(N_CORES):
        m = (np.arange(B_TOTAL) % N_CORES) == c
        hs, es = _pack_core(preds[m], ev[m], idx[m])
        in_maps.append({"hsort": hs, "esort": es})

    res = run_bass_kernel_spmd(nc, in_maps, core_ids=list(range(N_CORES)))
    _CACHE["last_results"] = res
    return _combine([r["partials"] for r in res.results], float(B_TOTAL))


if __name__ == "__main__":
    pass



# revision 9
# speedup vs baseline: 2.0049x; 2.0049x over previous
"""Trainium2 Bass kernel for nn_LossTDSurv (survival loss over hazards).

Strategy (v2): the loss is row-permutation invariant and only ever reads
columns 0..idx of each row, so the host sorts rows by idx and ships, in
bf16, just the needed prefix q_k = 1-h_k of every row plus four side
columns (h_idx, event, q_{idx-1}, q_idx).  All per-row ragged sums become
products of a compile-time-constant column prefix:

    prodA = prod_{k<=v-2} q_k      (one tensor_reduce(mult) per slot)
    A     = ln(prodA)              cond_sum
    logWt = ln(clip(1-prodA,1e-8)) (no exp/log roundtrip!)
    C_sum = sum A + sum ln(q_{v-1} q_v)

The ACT Ln spline saturates below ~1e-19 while prodA legitimately
reaches e^-87, so the host packs q' = 2q (exact in bf16) making the
device product prodA' = 2^(w-2) * prodA, always in Ln's accurate range.
The host subtracts the exact ln2 corrections from s_a / s_eA, and the
logWt path recovers unscaled prodA with a fused 1 - 2^-(w-2)*x
tensor_scalar per slot.

Group-to-core mapping: core c takes the 8 idx-groups {8s+c | s even} u
{8s+7-c | s odd}, exactly one per width-8 octave band, so EVERY core runs
the identical program with 8 fixed slot widths W_s = 8(s+1).  Groups are
right-aligned in their slot and padded with q=1.0 (multiplicative
identity), which keeps the reduce prefix [0, W_s-2) correct for every v.

The whole kernel is ~30 instructions: 10 DMAs, 8 DVE product-reduces, a
handful of elementwise ops split across DVE/Pool, 6 ACT ops with free
row-sum accumulation.  Per-core HBM traffic ~5 MB (vs 19 MB for the f32
full-width variant).

Per-core output: [128, 8] fp32 partial sums; host combines in float64.
"""

import numpy as np

B_TOTAL = 524288
T = 64
N_CORES = 8
NSLOT = 8
WIDTHS = [8 * (s + 1) for s in range(NSLOT)]   # 8,16,...,64
CLIP_WT = 1e-8
CLIP_PA = 1e-16   # on the 2^(w-2)-scaled product, inside Ln's good range

_CACHE = {}


def _build_nc(jb):
    """Single-core SPMD Bass program (same NEFF on all 8 cores)."""
    import concourse.bacc as bacc
    import concourse.mybir as mybir
    import concourse.tile as tile

    f32 = mybir.dt.float32
    bf16 = mybir.dt.bfloat16
    AF = mybir.ActivationFunctionType
    OP = mybir.AluOpType
    AX = mybir.AxisListType

    nb = NSLOT * jb                    # per-lane row-buffer width
    wsum = sum(WIDTHS)                 # 288

    nc = bacc.Bacc("TRN2", target_bir_lowering=False, debug=False)

    qpack = nc.dram_tensor("qpack", [128, jb * wsum], bf16, kind="ExternalInput")
    side = nc.dram_tensor("side", [128, 4 * nb], bf16, kind="ExternalInput")
    partials = nc.dram_tensor("partials", [128, 8], f32, kind="ExternalOutput")

    with tile.TileContext(nc) as tc:
        with (
            tc.tile_pool(name="io", bufs=3) as io,
            tc.tile_pool(name="pers", bufs=1) as pers,
        ):
            sd = pers.tile([128, 4 * nb], bf16, tag="sd")
            Ab = pers.tile([128, nb], f32, tag="Ab")
            T2 = pers.tile([128, nb], f32, tag="T2")
            Aln = pers.tile([128, nb], f32, tag="Aln")
            logwt = pers.tile([128, nb], f32, tag="logwt")
            loghv = pers.tile([128, nb], f32, tag="loghv")
            lgv = pers.tile([128, nb], f32, tag="lgv")
            Qb = pers.tile([128, nb], bf16, tag="Qb")
            scr = pers.tile([128, nb], bf16, tag="scr")
            scr2 = pers.tile([128, nb], bf16, tag="scr2")
            acc = pers.tile([128, 8], f32, tag="acc")

            nc.sync.dma_start(sd[:], side[:])
            Hv = sd[:, 0:nb]
            Eb = sd[:, nb:2 * nb]
            Qm1 = sd[:, 2 * nb:3 * nb]
            Qv = sd[:, 3 * nb:4 * nb]

            # --- early work off the side tensor (overlaps main loop) ---
            # first ACT op also triggers the Ln table load
            nc.scalar.activation(loghv[:], Hv, AF.Ln)
            nc.scalar.activation(lgv[:], Hv, AF.Ln, bias=1.0, scale=-1.0)
            nc.gpsimd.tensor_tensor(out=Qb[:], in0=Qm1, in1=Qv, op=OP.mult)
            nc.scalar.activation(scr[:], Qb[:], AF.Ln,
                                 accum_out=acc[:, 1:2])          # s_cq
            nc.scalar.activation(scr2[:], Eb, AF.Copy,
                                 accum_out=acc[:, 2:3])          # s_e
            # only the e-weighted sums of loghv/lgv are needed downstream,
            # so overwriting them in place is fine
            nc.vector.scalar_tensor_tensor(
                out=loghv[:], in0=loghv[:], scalar=0.0, in1=Eb,
                op0=OP.add, op1=OP.mult, accum_out=acc[:, 4:5],
            )  # s_eloghv
            nc.vector.scalar_tensor_tensor(
                out=lgv[:], in0=lgv[:], scalar=0.0, in1=Eb,
                op0=OP.add, op1=OP.mult, accum_out=acc[:, 6:7],
            )  # s_elgv
            nc.vector.memset(acc[:, 7:8], 0.0)

            # --- main loop: one DMA + one product-reduce per slot ---
            off = 0
            for s in range(NSLOT):
                w = WIDTHS[s]
                Wt = io.tile([128, jb * w], bf16, tag="W")
                nc.sync.dma_start(Wt[:], qpack[:, off:off + jb * w])
                w3 = Wt[:].rearrange("p (j w) -> p j w", w=w)
                nc.vector.tensor_reduce(
                    Ab[:, s * jb:(s + 1) * jb], w3[:, :, :w - 2],
                    axis=AX.X, op=OP.mult,
                )
                off += jb * w

            # --- epilogue ---
            for s in range(NSLOT):
                w = WIDTHS[s]
                sl = np.s_[:, s * jb:(s + 1) * jb]
                nc.vector.tensor_scalar(
                    out=T2[sl], in0=Ab[sl], scalar1=-(2.0 ** -(w - 2)),
                    scalar2=1.0, op0=OP.mult, op1=OP.add,
                )  # 1 - prodA (unscaled)
            nc.vector.tensor_scalar_max(out=T2[:], in0=T2[:], scalar1=CLIP_WT)
            nc.vector.tensor_scalar_max(out=Ab[:], in0=Ab[:], scalar1=CLIP_PA)
            nc.scalar.activation(Aln[:], Ab[:], AF.Ln,
                                 accum_out=acc[:, 0:1])          # s_a
            nc.scalar.activation(logwt[:], T2[:], AF.Ln)
            nc.vector.scalar_tensor_tensor(
                out=Aln[:], in0=Aln[:], scalar=0.0, in1=Eb,
                op0=OP.add, op1=OP.mult, accum_out=acc[:, 3:4],
            )  # s_eA
            nc.vector.scalar_tensor_tensor(
                out=logwt[:], in0=logwt[:], scalar=0.0, in1=Eb,
                op0=OP.add, op1=OP.mult, accum_out=acc[:, 5:6],
            )  # s_elogwt

            nc.sync.dma_start(partials[:], acc[:])

    nc.finalize()
    return nc


def _core_groups(c):
    out = []
    for s in range(NSLOT):
        out.append(8 * s + c if s % 2 == 0 else 8 * s + 7 - c)
    return out


def _pack_core(c, q, preds, ev, rows_by_group, jb):
    """Pack one core's 8 groups into qpack [128, jb*288] bf16 and
    side [128, 4*8*jb] bf16 (hv | e | q_{v-1} | q_v)."""
    import ml_dtypes

    bf = ml_dtypes.bfloat16
    gr = 128 * jb
    nb = NSLOT * jb
    qblocks = []
    hv_all = np.full((128, nb), 0.5, np.float32)
    e_all = np.zeros((128, nb), np.float32)
    qm1_all = np.ones((128, nb), np.float32)
    qv_all = np.ones((128, nb), np.float32)

    for s, v in enumerate(_core_groups(c)):
        w = WIDTHS[s]
        rows = rows_by_group[v]
        n = len(rows)
        assert n <= gr, f"group {v} overflow: {n} > {gr}"
        blk = np.full((gr, w), 2.0, np.float32)
        blk[:n, w - v - 1:] = 2.0 * q[rows, :v + 1]
        qblocks.append(blk.reshape(128, jb * w))

        col = np.s_[:, s * jb:(s + 1) * jb]
        hv = np.full(gr, 0.5, np.float32)
        hv[:n] = preds[rows, v]
        hv_all[col] = hv.reshape(128, jb)
        e = np.zeros(gr, np.float32)
        e[:n] = ev[rows]
        e_all[col] = e.reshape(128, jb)
        if v >= 1:
            qm1 = np.ones(gr, np.float32)
            qm1[:n] = q[rows, v - 1]
            qm1_all[col] = qm1.reshape(128, jb)
        qv_ = np.ones(gr, np.float32)
        qv_[:n] = q[rows, v]
        qv_all[col] = qv_.reshape(128, jb)

    qpack = np.ascontiguousarray(
        np.concatenate(qblocks, axis=1)).astype(bf)
    sidearr = np.ascontiguousarray(
        np.concatenate([hv_all, e_all, qm1_all, qv_all], axis=1)).astype(bf)
    return {"qpack": qpack, "side": sidearr}


def _combine(partials_list, b_total, corr_a, corr_eA):
    s = np.zeros(8, np.float64)
    for pcore in partials_list:
        s += pcore.astype(np.float64).sum(axis=0)
    s_a, s_cq, s_e, s_eA, s_eloghv, s_elogwt, s_elgv, _ = s
    s_a -= corr_a      # undo the 2^(w-2) product scaling: ln2*(w-2) per row
    s_eA -= corr_eA
    L_z = -(s_eloghv + s_eA) / s_e
    L_c = -(s_a - s_eA + s_elogwt) / b_total
    nll = -(s_a + s_cq + s_eloghv - s_elgv) / b_total
    return np.float32(0.5 * L_z + 0.5 * L_c + 1.0 * nll)


def kernel(preds: np.ndarray, target: np.ndarray) -> np.ndarray:
    from concourse.bass_utils import run_bass_kernel_spmd

    b_total = preds.shape[0]
    preds = np.asarray(preds, np.float32).reshape(b_total, T)
    target = np.asarray(target, np.float32).reshape(b_total, 3)
    idx = target[:, 0].astype(np.int64)
    ev = target[:, 1].astype(np.float32)
    q = (np.float32(1.0) - preds)

    counts = np.bincount(idx, minlength=T)
    jb = max(2, int(np.ceil(counts.max() / 128)))

    order = np.argsort(idx, kind="stable")
    bounds = np.cumsum(counts)[:-1]
    rows_by_group = np.split(order, bounds)

    if _CACHE.get("jb") != jb:
        _CACHE["nc"] = _build_nc(jb)
        _CACHE["jb"] = jb
    nc = _CACHE["nc"]

    in_maps = [
        _pack_core(c, q, preds, ev, rows_by_group, jb) for c in range(N_CORES)
    ]

    # exact corrections for the host-side 2x scaling of q
    w_row = 8.0 * (idx // 8 + 1)               # slot width of each row's group
    ln2 = float(np.log(2.0))
    # every padded slot row (gr per slot, incl dummies) contributes (w-2)*ln2
    corr_a = ln2 * N_CORES * 128 * jb * sum(w - 2 for w in WIDTHS)
    corr_eA = ln2 * float((ev.astype(np.float64) * (w_row - 2.0)).sum())

    res = run_bass_kernel_spmd(nc, in_maps, core_ids=list(range(N_CORES)))
    _CACHE["last_results"] = res
    return _combine([r["partials"] for r in res.results], float(b_total),
                    corr_a, corr_eA)


if __name__ == "__main__":
    pass
